# revision 40
# baseline (speedup 1.0000x reference)
"""GAT (2-layer, PyG-style) distributed Bass kernel for 8 Trainium2 NeuronCores.

Strategy (graph/data parallel, per sharding hint):
  - Nodes are partitioned into 8 contiguous blocks; core c owns destination
    nodes [c*N/8, (c+1)*N/8) and all edges incident to them. Self loops are
    applied analytically in the tile epilogue.
  - Each layer: every core builds the full node feature table
    tbl[v] = [xh(v) | e_src(v)] in bf16 (padded to a 512B/256B row), then for
    each destination tile a hardware dma_gather fetches the per-edge source
    rows, attention is formed with a fused leaky_relu + exp (scores are O(1)
    so exp never overflows; softmax is exactly equivalent without the max
    subtraction), and a 0/1 selection-matrix matmul on the tensor engine
    performs the per-destination segment reduction of [msg | ea] in PSUM.
  - All tensor-engine operands are bf16 (PSUM accumulation stays fp32);
    element-wise work is batched per destination tile (not per 128-edge
    chunk) to amortize per-instruction overheads; PSUM->SBUF copies run on
    the scalar (activation) engine to keep the vector engine free.
  - Host reassembles the transposed hidden table h_T (bf16) from the 8
    shards, then launch 2 repeats the same structure with 41-wide features
    for the single-head output layer.

SPMD constraints force fully uniform static structure across cores: every
(dst-tile x src-quarter) edge segment is padded to S chunks of 128 edges
(pad edges gather row 0 and use an out-of-range dst slot so selection
matrices zero them out). Source indices are split into 4 quarters because
dma_gather indices are int16.
"""

import math
import os
import sys

for _p in ("/opt/trn_rl_repo", "/root/.axon_site/_ro/trn_rl_repo"):
    if os.path.isdir(_p) and _p not in sys.path:
        sys.path.insert(0, _p)

import numpy as np
import ml_dtypes
from contextlib import ExitStack

import concourse.bacc as bacc
import concourse.bass as bass
import concourse.tile as tile
from concourse import mybir
from concourse.bass_utils import run_bass_kernel_spmd

F32 = mybir.dt.float32
BF16 = mybir.dt.bfloat16
I16 = mybir.dt.int16
AF = mybir.ActivationFunctionType
ALU = mybir.AluOpType

NEG_SLOPE = 0.2
EPS = 1e-16
P = 128
PAD_DST = 200.0  # sentinel dst_local for pad edges; never matches iota 0..127
MAXC = 4         # chunks per dma_gather call (HW-validated regime)
BARRIER_EVERY = 4


# --------------------------------------------------------------------------
# host-side graph preprocessing
# --------------------------------------------------------------------------

def _round_up(a, b):
    return (a + b - 1) // b * b


class EdgeStruct:
    """Uniform SPMD edge layout shared by both layers."""

    def __init__(self, src, dst, N, n_cores, G=3):
        self.N = N
        self.n_cores = n_cores
        self.G = G
        self.Npad = _round_up(N, 512)
        self.Qsz = self.Npad // 4
        assert self.Qsz <= 32767
        assert N % n_cores == 0
        self.npc = N // n_cores                      # dst nodes per core
        self.T = math.ceil(self.npc / P)             # real dst tiles per core
        self.T_pad = _round_up(self.T, G)
        self.n_groups = self.T_pad // G
        nseg = self.T_pad * 4

        src = src.astype(np.int64)
        dst = dst.astype(np.int64)

        per_core = []
        max_cnt = 0
        for c in range(n_cores):
            lo = c * self.npc
            sel = (dst >= lo) & (dst < lo + self.npc)
            s_c = src[sel]
            dl = dst[sel] - lo                        # local dst id
            t_all = dl >> 7                           # dst tile
            q_all = s_c // self.Qsz                   # src quarter
            key = t_all * 4 + q_all
            order = np.argsort(key, kind="stable")
            s_c, dl, key = s_c[order], dl[order], key[order]
            cnt = np.bincount(key, minlength=nseg)
            max_cnt = max(max_cnt, int(cnt.max()))
            per_core.append((s_c, dl, key, cnt))

        self.S = max(1, math.ceil(max_cnt / P))      # chunks per segment
        S, G_, Qsz = self.S, G, self.Qsz
        self.ncols = 4 * G * S                       # chunk columns per group
        assert self.ncols <= P, f"ncols={self.ncols} > 128; lower S or G"
        slots_seg = S * P

        self.gidx = []    # [n_groups*4*128, G*S*8] int16
        self.gdl = []     # [n_groups*128, ncols]   bf16
        for c in range(n_cores):
            s_c, dl, key, cnt = per_core[c]
            flat_idx = np.zeros(nseg * slots_seg, np.int16)
            flat_dl = np.full(nseg * slots_seg, PAD_DST, np.float32)
            starts = np.concatenate([[0], np.cumsum(cnt)])[:-1]
            pos_in_seg = np.arange(len(s_c)) - starts[key]
            base = key * slots_seg
            pos = base + pos_in_seg
            q_of_edge = key % 4
            flat_idx[pos] = (s_c - q_of_edge * Qsz).astype(np.int16)
            flat_dl[pos] = (dl & 127).astype(np.float32)

            # flat layout is segment-major: seg = t*4+q, inside: s*128+p.
            # regroup to gather order: per (g, q): (t_loc, s, p)
            fi = flat_idx.reshape(self.T_pad, 4, S, P)
            fd = flat_dl.reshape(self.T_pad, 4, S, P)
            fi = fi.reshape(self.n_groups, G_, 4, S, P).transpose(0, 2, 1, 3, 4)
            fd = fd.reshape(self.n_groups, G_, 4, S, P).transpose(0, 2, 1, 3, 4)

            # gather idx arrays: flat i = (t_loc*S+s)*128+p ; wrapped [128, i/16]
            fi2 = fi.reshape(self.n_groups, 4, G_ * S * P)
            w = fi2.reshape(self.n_groups, 4, G_ * S * 8, 16)
            w = np.transpose(w, (0, 1, 3, 2))              # [g, 4, 16, cols16]
            w = np.tile(w, (1, 1, 8, 1))                   # replicate to 128
            self.gidx.append(
                np.ascontiguousarray(w.reshape(self.n_groups * 4 * P, G_ * S * 8))
            )

            # dst_local per slot: col c = q*(G*S)+t_loc*S+s
            fcol = fd.reshape(self.n_groups, self.ncols, P)   # [g, c, p]
            gdl = np.transpose(fcol, (0, 2, 1))               # [g, p, c]
            self.gdl.append(
                np.ascontiguousarray(
                    gdl.reshape(self.n_groups * P, self.ncols)
                ).astype(ml_dtypes.bfloat16)
            )


# --------------------------------------------------------------------------
# device kernel builder (shared by both layers)
# --------------------------------------------------------------------------

def build_layer_kernel(es: EdgeStruct, layer: int):
    """layer 1: tbl row [xh1(128)|e_src1(8)|pad], 256 bf16 = 512B,
               heads=8, csz=16, epilogue = softmax-div + ELU + transpose out.
       layer 2: row [xh2(40)|e_src2(1)|pad], 128 bf16 = 256B, heads=1,
               csz=40, epilogue = softmax-div, row-major f32 out."""
    Npad, T_pad, G, S, ncols = es.Npad, es.T_pad, es.G, es.S, es.ncols
    n_groups, Qsz = es.n_groups, es.Qsz
    if layer == 1:
        ELEM, H, CSZ, WCOLS = 256, 8, 16, 136
    else:
        ELEM, H, CSZ, WCOLS = 128, 1, 40, 41
    MW = H * CSZ                      # message width (128 / 40)
    AW = MW + H                       # [msg | ea] width (136 / 41)
    NCH = 4 * S                       # chunks per tile

    nc = bacc.Bacc("TRN2", target_bir_lowering=False, debug=False,
                   num_devices=es.n_cores)
    ap = {}
    ap["xT"] = nc.dram_tensor("xT", [P, Npad], BF16, kind="ExternalInput").ap()
    ap["xTm"] = nc.dram_tensor("xTm", [P, T_pad * P], BF16,
                               kind="ExternalInput").ap()
    ap["wext"] = nc.dram_tensor("wext", [P, WCOLS], BF16,
                                kind="ExternalInput").ap()
    ap["bmat"] = nc.dram_tensor("bmat", [P, MW], BF16,
                                kind="ExternalInput").ap()
    ap["wdst"] = nc.dram_tensor("wdst", [P, H], BF16, kind="ExternalInput").ap()
    ap["gidx"] = nc.dram_tensor("gidx", [n_groups * 4 * P, G * S * 8], I16,
                                kind="ExternalInput").ap()
    ap["gdl"] = nc.dram_tensor("gdl", [n_groups * P, ncols], BF16,
                               kind="ExternalInput").ap()
    ap["iota_bf"] = nc.dram_tensor("iota_bf", [P, P], BF16,
                                   kind="ExternalInput").ap()
    ap["idn"] = nc.dram_tensor("idn", [P, P], BF16, kind="ExternalInput").ap()
    if layer == 1:
        out_ap = nc.dram_tensor("hT", [P, T_pad * P], BF16,
                                kind="ExternalOutput").ap()
    else:
        out_ap = nc.dram_tensor("logits", [T_pad * P, CSZ], F32,
                                kind="ExternalOutput").ap()
    # +128 guard rows so 256-elem reads from the last quarter stay in bounds
    tbl = nc.dram_tensor("tbl", [Npad + P, ELEM], BF16, kind="Internal").ap()
    own_tbl = nc.dram_tensor("own_tbl", [T_pad * P, WCOLS], BF16,
                             kind="Internal").ap()

    with tile.TileContext(nc) as tc, ExitStack() as ctx:
        cpool = ctx.enter_context(tc.tile_pool(name="consts", bufs=1))

        # ---- constants ----
        wext = cpool.tile([P, WCOLS], BF16)
        nc.sync.dma_start(wext[:], ap["wext"])
        bmat = cpool.tile([P, MW], BF16)
        nc.sync.dma_start(bmat[:], ap["bmat"])
        wdst = cpool.tile([P, H], BF16)
        nc.sync.dma_start(wdst[:], ap["wdst"])
        iota_bf = cpool.tile([P, P], BF16)
        nc.sync.dma_start(iota_bf[:], ap["iota_bf"])
        idn = cpool.tile([P, P], BF16)
        nc.sync.dma_start(idn[:], ap["idn"])
        edst_sb = cpool.tile([P, T_pad * H], BF16)

        with tc.tile_pool(name="pre_sb", bufs=6) as psb, \
                tc.tile_pool(name="pre_ps", bufs=4, space="PSUM") as pps, \
                tc.tile_pool(name="pre_ps2", bufs=2, space="PSUM") as pps2:
            # ---- pre-pass A: full feature table, 4 node-tiles per
            # iteration to amortize DMA dispatch (bias applied in the tile
            # epilogue, not here) ----
            for i in range(Npad // (4 * P)):
                xt4 = psb.tile([P, 4 * P], BF16, tag="xt")
                nc.scalar.dma_start(xt4[:],
                                    ap["xT"][:, i * 4 * P:(i + 1) * 4 * P])
                ot4 = psb.tile([P, 4 * WCOLS], BF16, tag="ot")
                for j in range(2):
                    pp2 = pps.tile([P, 2 * WCOLS], F32, tag="pp2")
                    for k in range(2):
                        nc.tensor.matmul(
                            out=pp2[:, k * WCOLS:(k + 1) * WCOLS],
                            lhsT=xt4[:, (2 * j + k) * P:(2 * j + k + 1) * P],
                            rhs=wext[:], start=True, stop=True,
                            skip_group_check=True)
                    if j == 0:
                        nc.scalar.copy(out=ot4[:, 0:2 * WCOLS], in_=pp2[:])
                    else:
                        nc.vector.tensor_copy(
                            out=ot4[:, 2 * WCOLS:4 * WCOLS], in_=pp2[:])
                dst4 = tbl[i * 4 * P:(i + 1) * 4 * P, 0:WCOLS] \
                    .rearrange("(b p) c -> p b c", p=P)
                nc.sync.dma_start(
                    dst4, ot4[:].rearrange("p (b c) -> p b c", c=WCOLS))

            # ---- pre-pass B: own-node rows [xh|e_src] (DRAM) and e_dst
            # (SBUF-resident) ----
            for t in range(T_pad):
                xt = psb.tile([P, P], BF16, tag="xt2")
                nc.sync.dma_start(xt[:], ap["xTm"][:, t * P:(t + 1) * P])
                po = pps2.tile([P, WCOLS], F32, tag="po")
                nc.tensor.matmul(out=po[:], lhsT=xt[:], rhs=wext[:],
                                 start=True, stop=True, skip_group_check=True)
                oo = psb.tile([P, WCOLS], BF16, tag="oo")
                nc.scalar.copy(out=oo[:], in_=po[:])
                nc.sync.dma_start(own_tbl[t * P:(t + 1) * P, :], oo[:])
                pe = pps2.tile([P, H], F32, tag="pe")
                nc.tensor.matmul(out=pe[:], lhsT=xt[:], rhs=wdst[:],
                                 start=True, stop=True)
                nc.vector.tensor_copy(out=edst_sb[:, t * H:(t + 1) * H],
                                      in_=pe[:])

        # ---- edge pass ----
        sb = ctx.enter_context(tc.tile_pool(name="sb", bufs=2))
        tpool = ctx.enter_context(tc.tile_pool(name="tp", bufs=2))
        gbp = ctx.enter_context(tc.tile_pool(name="gbuf", bufs=2))
        trp = ctx.enter_context(tc.tile_pool(name="trp", bufs=2, space="PSUM"))
        edp = ctx.enter_context(tc.tile_pool(name="edp", bufs=1, space="PSUM"))
        pacc = ctx.enter_context(tc.tile_pool(name="pacc", bufs=2,
                                              space="PSUM"))
        ptp = ctx.enter_context(tc.tile_pool(name="ptp", bufs=1, space="PSUM"))
        def load_group(g):
            """Metadata DMAs for group g: gather indices, dst-locals,
            own-node rows. Called one group ahead so the loads land
            before the preceding barrier stalls the sync engine."""
            idxs = sb.tile([P, 4 * G * S * 8], I16, tag="idx")
            nc.sync.dma_start(
                idxs[:].rearrange("p (q w) -> p q w", w=G * S * 8),
                ap["gidx"][g * 4 * P:(g + 1) * 4 * P, :]
                .rearrange("(q p) w -> p q w", p=P))
            dlt = sb.tile([P, ncols], BF16, tag="dl")
            nc.sync.dma_start(dlt[:], ap["gdl"][g * P:(g + 1) * P, :])
            own_g = sb.tile([P, G * WCOLS], BF16, tag="own")
            nc.sync.dma_start(
                own_g[:].rearrange("p (b c) -> p b c", c=WCOLS),
                own_tbl[g * G * P:(g + 1) * G * P, :]
                .rearrange("(b p) c -> p b c", p=P))
            return idxs, dlt, own_g

        # No initial barrier: the gathers' RAW deps on tbl (tracked DRAM
        # accesses) order their transfers after pre-pass A's writes, so
        # pre-pass B overlaps the first gather window's descriptor
        # generation. Mid-loop barriers below still bound in-flight
        # SWDGE descriptors.
        cur = load_group(0)
        for g in range(n_groups):
            if g % BARRIER_EVERY == 0 and g > 0:
                tc.strict_bb_all_engine_barrier()
            idxs, dlt, own_g = cur
            nxt = load_group(g + 1) if g + 1 < n_groups else None
            gb = gbp.tile([P, ncols * ELEM], BF16, tag="gb")
            gb3 = gb[:].rearrange("p (c k) -> p c k", k=ELEM)
            for q in range(4):
                for c0 in range(0, G * S, MAXC):
                    c1 = min(c0 + MAXC, G * S)
                    nc.gpsimd.dma_gather(
                        out_ap=gb3[:, q * G * S + c0:q * G * S + c1, :],
                        in_ap=tbl[q * Qsz:q * Qsz + Qsz, :],
                        idxs_ap=idxs[:, (q * G * S + c0) * 8:
                                     (q * G * S + c1) * 8],
                        num_idxs=(c1 - c0) * P,
                        num_idxs_reg=(c1 - c0) * P,
                        elem_size=ELEM,
                    )
            for t_loc in range(G):
                t = g * G + t_loc

                # ---- selection matrices for all chunks of this tile ----
                seT = tpool.tile([P, NCH * P], BF16, tag="seT")
                se3 = seT[:].rearrange("p (c d) -> p c d", d=P)
                for q in range(4):
                    dsl = dlt[:, q * G * S + t_loc * S:
                              q * G * S + t_loc * S + S]
                    nc.vector.tensor_tensor(
                        out=se3[:, q * S:(q + 1) * S, :],
                        in0=dsl.rearrange("p (s o) -> p s o", o=1)
                        .to_broadcast([P, S, P]),
                        in1=iota_bf[:].rearrange("p (o d) -> p o d", o=1)
                        .to_broadcast([P, S, P]),
                        op=ALU.is_equal)

                # ---- transpose to S_dT (PSUM, bf16) then copy to SBUF.
                # Two half-batches double-buffer the transpose PSUM; the
                # copies alternate between scalar and vector engines. ----
                sdT = tpool.tile([P, NCH * P], BF16, tag="sdT")
                h0 = NCH // 2
                for half, (c_lo, c_hi) in enumerate(((0, h0), (h0, NCH))):
                    trs = trp.tile([P, h0 * P], BF16, tag="trs")
                    for cq in range(c_lo, c_hi):
                        o = (cq - c_lo) * P
                        nc.tensor.matmul(out=trs[:, o:o + P],
                                         lhsT=seT[:, cq * P:(cq + 1) * P],
                                         rhs=idn[:], is_transpose=True,
                                         start=True, stop=True,
                                         skip_group_check=True)
                    half_w = (c_hi - c_lo) * P
                    for jj, j in enumerate(range(0, half_w, 4 * P)):
                        j1 = min(j + 4 * P, half_w)
                        dst = sdT[:, c_lo * P + j:c_lo * P + j1]
                        if (half + jj) % 2 == 0:
                            nc.scalar.copy(out=dst, in_=trs[:, j:j1])
                        else:
                            nc.vector.tensor_copy(out=dst, in_=trs[:, j:j1])

                # ---- e_dst per edge: one matmul per chunk into one PSUM ----
                edps = edp.tile([P, NCH * H], F32, tag="ed")
                for cq in range(NCH):
                    nc.tensor.matmul(
                        out=edps[:, cq * H:(cq + 1) * H],
                        lhsT=sdT[:, cq * P:(cq + 1) * P],
                        rhs=edst_sb[:, t * H:(t + 1) * H],
                        start=True, stop=True, skip_group_check=True)

                # ---- attention: alpha = lrelu(es + ed); ea = exp(alpha) ----
                mea = tpool.tile([P, NCH * AW], BF16, tag="mea")
                mea3 = mea[:].rearrange("p (c w) -> p c w", w=AW)
                al = sb.tile([P, NCH * H], F32, tag="al")
                al3 = al[:].rearrange("p (c h) -> p c h", h=H)
                for q in range(4):
                    base = q * G * S + t_loc * S
                    nc.vector.tensor_tensor(
                        out=al3[:, q * S:(q + 1) * S, :],
                        in0=gb3[:, base:base + S, MW:MW + H],
                        in1=edps[:].rearrange("p (c h) -> p c h", h=H)
                        [:, q * S:(q + 1) * S, :],
                        op=ALU.add)
                # fused leaky_relu: max(a, 0.2*a)
                nc.vector.scalar_tensor_tensor(
                    out=al[:], in0=al[:], scalar=NEG_SLOPE, in1=al[:],
                    op0=ALU.mult, op1=ALU.max)
                ea = sb.tile([P, NCH * H], BF16, tag="ea")
                nc.scalar.activation(out=ea[:], in_=al[:], func=AF.Exp)
                nc.vector.tensor_copy(
                    out=mea3[:, :, MW:AW],
                    in_=ea[:].rearrange("p (c h) -> p c h", h=H))

                # ---- messages: msg = ea * xh (broadcast over channels) ----
                for q in range(4):
                    base = q * G * S + t_loc * S
                    if H == 1:
                        nc.vector.tensor_tensor(
                            out=mea3[:, q * S:(q + 1) * S, 0:MW],
                            in0=mea3[:, q * S:(q + 1) * S, MW:AW]
                            .to_broadcast([P, S, MW]),
                            in1=gb3[:, base:base + S, 0:MW],
                            op=ALU.mult)
                    else:
                        msl = mea3[:, q * S:(q + 1) * S, :]
                        nc.vector.tensor_tensor(
                            out=msl[:, :, 0:MW].rearrange(
                                "p s (h k) -> p s h k", k=CSZ),
                            in0=msl[:, :, MW:AW].rearrange(
                                "p s (h o) -> p s h o", o=1)
                            .to_broadcast([P, S, H, CSZ]),
                            in1=gb3[:, base:base + S, 0:MW].rearrange(
                                "p s (h k) -> p s h k", k=CSZ),
                            op=ALU.mult)

                # ---- segment-reduce into the tile accumulator ----
                acc = pacc.tile([P, AW], F32, tag="acc")
                for cq in range(NCH):
                    nc.tensor.matmul(out=acc[:],
                                     lhsT=seT[:, cq * P:(cq + 1) * P],
                                     rhs=mea[:, cq * AW:(cq + 1) * AW],
                                     start=(cq == 0), stop=(cq == NCH - 1),
                                     skip_group_check=True)

                # ---- tile epilogue (adds analytic self-loop term) ----
                own = own_g[:, t_loc * WCOLS:(t_loc + 1) * WCOLS]
                als = sb.tile([P, H], F32, tag="als")
                nc.vector.tensor_tensor(out=als[:], in0=own[:, MW:WCOLS],
                                        in1=edst_sb[:, t * H:(t + 1) * H],
                                        op=ALU.add)
                nc.vector.scalar_tensor_tensor(
                    out=als[:], in0=als[:], scalar=NEG_SLOPE, in1=als[:],
                    op0=ALU.mult, op1=ALU.max)
                eas = sb.tile([P, H], F32, tag="eas")
                nc.scalar.activation(out=eas[:], in_=als[:], func=AF.Exp)
                smsg = sb.tile([P, MW], F32, tag="smsg")
                if H == 1:
                    nc.vector.tensor_tensor(
                        out=smsg[:], in0=eas[:, 0:1].to_broadcast([P, MW]),
                        in1=own[:, 0:MW], op=ALU.mult)
                else:
                    nc.vector.tensor_tensor(
                        out=smsg[:].rearrange("p (h c) -> p h c", c=CSZ),
                        in0=eas[:].rearrange("p (h o) -> p h o", o=1)
                        .to_broadcast([P, H, CSZ]),
                        in1=own[:, 0:MW].rearrange("p (h c) -> p h c", c=CSZ),
                        op=ALU.mult)
                unorm = sb.tile([P, MW], F32, tag="unorm")
                nc.vector.tensor_tensor(out=unorm[:], in0=acc[:, 0:MW],
                                        in1=smsg[:], op=ALU.add)
                den = sb.tile([P, H], F32, tag="den")
                # (acc_ea + EPS) + eas in one fused op
                nc.vector.scalar_tensor_tensor(
                    out=den[:], in0=acc[:, MW:AW], scalar=EPS, in1=eas[:],
                    op0=ALU.add, op1=ALU.add)
                rec = sb.tile([P, H], F32, tag="rec")
                nc.vector.reciprocal(out=rec[:], in_=den[:])
                # bias: out = unorm/den + b  ==  (unorm + b*den)/den
                btmp = sb.tile([P, MW], F32, tag="btmp")
                if H == 1:
                    nc.vector.tensor_tensor(
                        out=btmp[:], in0=den[:, 0:1].to_broadcast([P, MW]),
                        in1=bmat[:], op=ALU.mult)
                else:
                    nc.vector.tensor_tensor(
                        out=btmp[:].rearrange("p (h c) -> p h c", c=CSZ),
                        in0=den[:].rearrange("p (h o) -> p h o", o=1)
                        .to_broadcast([P, H, CSZ]),
                        in1=bmat[:].rearrange("p (h c) -> p h c", c=CSZ),
                        op=ALU.mult)
                nc.vector.tensor_tensor(out=unorm[:], in0=unorm[:],
                                        in1=btmp[:], op=ALU.add)
                otile = sb.tile([P, MW], F32, tag="otile")
                if H == 1:
                    nc.vector.tensor_tensor(
                        out=otile[:], in0=rec[:, 0:1].to_broadcast([P, MW]),
                        in1=unorm[:], op=ALU.mult)
                else:
                    rec3 = rec[:].rearrange("p (h o) -> p h o", o=1) \
                        .to_broadcast([P, H, CSZ])
                    acc3 = unorm[:].rearrange("p (h c) -> p h c", c=CSZ)
                    ot3 = otile[:].rearrange("p (h c) -> p h c", c=CSZ)
                    nc.vector.tensor_tensor(out=ot3, in0=rec3, in1=acc3,
                                            op=ALU.mult)
                if layer == 1:
                    # ELU then transpose out (bf16)
                    tmp = sb.tile([P, MW], F32, tag="tmp")
                    nc.vector.tensor_scalar_min(out=tmp[:], in0=otile[:],
                                                scalar1=0.0)
                    nc.scalar.activation(out=tmp[:], in_=tmp[:], func=AF.Exp)
                    nc.scalar.activation(out=otile[:], in_=otile[:],
                                         func=AF.Relu)
                    obf = sb.tile([P, MW], BF16, tag="obf")
                    # (tmp - 1) + relu(x) fused
                    nc.vector.scalar_tensor_tensor(
                        out=obf[:], in0=tmp[:], scalar=-1.0, in1=otile[:],
                        op0=ALU.add, op1=ALU.add)
                    tp = ptp.tile([P, P], BF16, tag="tp")
                    nc.tensor.transpose(out=tp[:], in_=obf[:],
                                        identity=idn[:])
                    hTt = sb.tile([P, P], BF16, tag="hTt")
                    nc.scalar.copy(out=hTt[:], in_=tp[:])
                    nc.sync.dma_start(out_ap[:, t * P:(t + 1) * P], hTt[:])
                else:
                    nc.sync.dma_start(out_ap[t * P:(t + 1) * P, :], otile[:])
            cur = nxt

    nc.compile()
    return nc


# --------------------------------------------------------------------------
# host orchestration
# --------------------------------------------------------------------------

def _consts_inputs():
    iota = np.arange(P, dtype=np.float32)
    return {
        "iota_bf": np.tile(iota.astype(ml_dtypes.bfloat16)[None, :], (P, 1)),
        "idn": np.eye(P, dtype=ml_dtypes.bfloat16),
    }


def _blockdiag(att):
    """[H, C] attention vector -> [H*C, H] block-diagonal matrix."""
    H, C = att.shape
    out = np.zeros((H * C, H), np.float32)
    for h in range(H):
        out[h * C:(h + 1) * C, h] = att[h]
    return out


def run_gat(x, edge_index, W1, att_src1, att_dst1, b1, W2, att_src2, att_dst2,
            b2, N, n_cores, G=3, es=None, verbose=False):
    x = np.asarray(x, np.float32)
    src = np.asarray(edge_index[0]).astype(np.int64)
    dst = np.asarray(edge_index[1]).astype(np.int64)
    # self-loops are handled analytically inside the kernel epilogue

    if es is None:
        es = EdgeStruct(src, dst, N, n_cores, G=G)
    npc, Npad, T_pad = es.npc, es.Npad, es.T_pad

    consts = _consts_inputs()
    xT = np.zeros((P, Npad), ml_dtypes.bfloat16)
    xT[:, :N] = np.asarray(x, np.float32).T.astype(ml_dtypes.bfloat16)

    W1 = np.asarray(W1, np.float32)
    w1ext = np.concatenate(
        [W1, W1 @ _blockdiag(np.asarray(att_src1, np.float32))], axis=1)
    w1dst = W1 @ _blockdiag(np.asarray(att_dst1, np.float32))
    bmat1 = np.tile(np.asarray(b1, np.float32)[None, :], (P, 1))

    nc1 = build_layer_kernel(es, 1)
    in_maps = []
    for c in range(n_cores):
        xTm = np.zeros((P, T_pad * P), ml_dtypes.bfloat16)
        xTm[:, :npc] = xT[:, c * npc:(c + 1) * npc]
        in_maps.append({
            "xT": xT, "xTm": xTm,
            "wext": w1ext.astype(ml_dtypes.bfloat16),
            "wdst": w1dst.astype(ml_dtypes.bfloat16),
            "bmat": bmat1.astype(ml_dtypes.bfloat16),
            "gidx": es.gidx[c], "gdl": es.gdl[c],
            **consts,
        })
    res1 = run_bass_kernel_spmd(nc1, in_maps, core_ids=list(range(n_cores)))
    hT = np.zeros((P, Npad), ml_dtypes.bfloat16)
    for c in range(n_cores):
        hT[:, c * npc:(c + 1) * npc] = res1.results[c]["hT"][:, :npc]

    W2 = np.asarray(W2, np.float32)
    w2ext = np.concatenate(
        [W2, W2 @ _blockdiag(np.asarray(att_src2, np.float32))], axis=1)
    w2dst = W2 @ _blockdiag(np.asarray(att_dst2, np.float32))
    bmat2 = np.tile(np.asarray(b2, np.float32)[None, :], (P, 1))

    nc2 = build_layer_kernel(es, 2)
    in_maps2 = []
    for c in range(n_cores):
        hTm = np.zeros((P, T_pad * P), ml_dtypes.bfloat16)
        hTm[:, :npc] = hT[:, c * npc:(c + 1) * npc]
        in_maps2.append({
            "xT": hT, "xTm": hTm,
            "wext": w2ext.astype(ml_dtypes.bfloat16),
            "wdst": w2dst.astype(ml_dtypes.bfloat16),
            "bmat": bmat2.astype(ml_dtypes.bfloat16),
            "gidx": es.gidx[c], "gdl": es.gdl[c],
            **consts,
        })
    res2 = run_bass_kernel_spmd(nc2, in_maps2, core_ids=list(range(n_cores)))
    out = np.zeros((N, 40), np.float32)
    for c in range(n_cores):
        out[c * npc:(c + 1) * npc] = res2.results[c]["logits"][:npc, :]
    return out


def kernel(x, edge_index, W1, att_src1, att_dst1, b1, W2, att_src2, att_dst2,
           b2):
    N = int(np.asarray(x).shape[0])
    return run_gat(x, edge_index, W1, att_src1, att_dst1, b1, W2, att_src2,
                   att_dst2, b2, N=N, n_cores=8)
